# revision 1
# baseline (speedup 1.0000x reference)
"""YOLOv1 loss (nn_LossModul_16277926052544) on 8 TRN2 NeuronCores.

Pure data parallel: batch 8192 -> 8 shards of 1024. Each core computes a
partial sum of the loss over its shard; host sums the 8x128 partials.

Math restructuring vs the reference (validated to 5.5e-07 rel err in numpy;
bf16 variant 1.9e-04):
  * IoU is translation invariant -> grid offsets cancel; overlap length per
    axis is min(pw, tw, (pw+tw)/2 - |c|) clamped to >= 0, c = (px-tx)/S.
  * resp = iou1 > iou2 <=> inter1*den2 > inter2*den1 (dens > 0).
  * Every loss term is a masked square; weights fold into masks or the
    Scalar engine's Square scale, so ACT Square+accum reduces everything.

Perf structure:
  * channel-planar SBUF tiles ([P, C, F], inner F contiguous) -- DVE pays a
    per-AP-row bubble, so interleaved-channel inner dims are ~8x slower.
  * bf16 intermediates -> DVE 2x_1P packing halves SBUF port traffic (the
    engines contend for the same ports, so total traffic is the limit).
  * DMAs cast f32->bf16 in flight (SWDGE) and are channel-split so the
    geometry pipeline starts after ~1.5MB instead of 6MB; the 5 unused
    target channels (5:10) are never transferred.
"""
import sys

for _p in ("/opt/trn_rl_repo",):
    if _p not in sys.path:
        sys.path.insert(0, _p)

import numpy as np
from contextlib import ExitStack

import concourse.bass as bass  # noqa: F401  (registers engines)
from concourse import bacc, mybir
from concourse import bass_utils
import concourse.tile as tile

N_CORES = 8
BATCH = 8192
S = 7
C = 30
TC = 25                                       # target channels kept: 0:5 + 10:30
P = 128
CELLS_PER_CORE = (BATCH // N_CORES) * S * S   # 50176
F_TOTAL = CELLS_PER_CORE // P                 # 392
T_TILES = 2
F = F_TOTAL // T_TILES                        # 196
R = 1.0 / S
EPS5 = 5e-6                                   # 5 * EPS (lambda folded)
SQRT5 = float(np.sqrt(5.0))
SQH = float(np.sqrt(0.5))

f32 = mybir.dt.float32
bf16 = mybir.dt.bfloat16
u32 = mybir.dt.uint32
Alu = mybir.AluOpType
Act = mybir.ActivationFunctionType

_CACHE = {}


def _build_body(tc, ctx, pred, targ, out_ap):
    nc = tc.nc
    inpool = ctx.enter_context(tc.tile_pool(name="in", bufs=2))
    wk = ctx.enter_context(tc.tile_pool(name="wk", bufs=2))
    stp = ctx.enter_context(tc.tile_pool(name="st", bufs=1))
    stats = stp.tile([P, 3 * T_TILES], f32)
    eps5c = stp.tile([P, 1], f32)               # bias const for Sqrt
    nc.gpsimd.memset(eps5c[:], EPS5)

    for t in range(T_TILES):
        # --- channel-split DMAs: boxes via HWDGE f32 (fast issue), cls via
        # SWDGE with f32->bf16 cast in flight
        ptile = inpool.tile([P, 10, F], f32, tag="p")
        ttile = inpool.tile([P, 5, F], f32, tag="t")
        pcls = inpool.tile([P, 20, F], bf16, tag="pc")
        tcls = inpool.tile([P, 20, F], bf16, tag="tc")
        nc.sync.dma_start(ttile[:], targ[:, t, 0:5])               # tar box
        nc.sync.dma_start(ptile[:], pred[:, t, 0:10])              # p boxes
        nc.gpsimd.dma_start(pcls[:], pred[:, t, 10:30])            # p cls
        nc.gpsimd.dma_start(tcls[:], targ[:, t, 10:30])            # tar cls

        pv = ptile[:]                                   # [P,10,F] f32
        tv = ttile[:]                                   # [P,5,F] f32
        pb = pv[:, 0:10, :].rearrange("p (b c) f -> p b c f", b=2)

        def bc(ap_pf, k):
            a = ap_pf
            if a.ndim == 2:
                a = a.unsqueeze(1)
            return a.broadcast_to([P, k, F])

        flat = lambda a: a.rearrange("p b c f -> p (b c f)")

        # ---------- geometry (bf16 intermediates) ----------
        dxy = wk.tile([P, 2, 2, F], f32, tag="dxy")    # [box, xy, f] raw p-t
        nc.gpsimd.tensor_sub(
            dxy[:], pb[:, :, 0:2, :],
            tv[:, 0:2, :].unsqueeze(1).broadcast_to([P, 2, 2, F]))
        sth = wk.tile([P, 2, F], bf16, tag="sth")      # 0.5*twh
        nc.vector.tensor_scalar_mul(sth[:], tv[:, 2:4, :], 0.5)
        s = wk.tile([P, 2, 2, F], bf16, tag="s")       # 0.5*pwh + 0.5*twh
        for b in range(2):                              # STT is <=3D
            nc.vector.scalar_tensor_tensor(
                s[:, b], pb[:, b, 2:4, :], 0.5, sth[:],
                op0=Alu.mult, op1=Alu.add)
        absd = wk.tile([P, 2, 2, F], bf16, tag="absd")  # |dxy|
        nc.scalar.activation(flat(absd[:]), flat(dxy[:]), Act.Abs)
        m = wk.tile([P, 2, 2, F], bf16, tag="m")       # s - R*|dxy|
        nc.vector.scalar_tensor_tensor(
            flat(m[:]), flat(absd[:]), -R, flat(s[:]),
            op0=Alu.mult, op1=Alu.add)
        minwh = wk.tile([P, 2, 2, F], bf16, tag="minwh")
        nc.vector.tensor_tensor(
            minwh[:], pb[:, :, 2:4, :],
            tv[:, 2:4, :].unsqueeze(1).broadcast_to([P, 2, 2, F]), op=Alu.min)
        ln = wk.tile([P, 2, 2, F], bf16, tag="ln")     # overlap lengths
        nc.vector.scalar_tensor_tensor(
            flat(ln[:]), flat(m[:]), 0.0, flat(minwh[:]),
            op0=Alu.max, op1=Alu.min)

        ID = wk.tile([P, 4, F], f32, tag="ID")         # [I1,I2,D1,D2]
        nc.vector.tensor_mul(ID[:, 0:2, :], ln[:, :, 0, :], ln[:, :, 1, :])
        nc.vector.tensor_mul(ID[:, 2:4, :], pv[:, 2:8:5, :], pv[:, 3:9:5, :])
        tarea = wk.tile([P, 1, F], f32, tag="tarea")
        nc.vector.tensor_mul(tarea[:], tv[:, 2:3, :], tv[:, 3:4, :])
        nc.gpsimd.tensor_sub(ID[:, 2:4, :], ID[:, 2:4, :], ID[:, 0:2, :])
        nc.gpsimd.tensor_add(ID[:, 2:4, :], ID[:, 2:4, :], bc(tarea[:], 2))

        g = wk.tile([P, 2, F], f32, tag="g")
        nc.vector.tensor_mul(g[:, 0, :], ID[:, 0, :], ID[:, 3, :])
        nc.vector.tensor_mul(g[:, 1, :], ID[:, 1, :], ID[:, 2, :])
        resp = wk.tile([P, F], u32, tag="resp")        # 1 -> box1
        nc.vector.tensor_tensor(resp[:], g[:, 0, :], g[:, 1, :], op=Alu.is_gt)

        # ---------- selects (box2 copied, box1 predicated over it) ----------
        sel = wk.tile([P, 7, F], f32, tag="sel")       # dx dy w h c n1 n2
        nc.scalar.copy(sel[:, 0:2, :], dxy[:, 1, :, :])
        nc.scalar.copy(sel[:, 2:5, :], pv[:, 7:10, :])
        nc.vector.copy_predicated(sel[:, 0:2, :], bc(resp[:], 2), dxy[:, 0, :, :])
        nc.vector.copy_predicated(sel[:, 2:5, :], bc(resp[:], 3), pv[:, 2:5, :])
        idsel = wk.tile([P, 2, F], f32, tag="idsel")   # [Isel, Dsel] f32
        nc.scalar.copy(idsel[:], ID[:, 1:4:2, :])
        nc.vector.copy_predicated(idsel[:], bc(resp[:], 2), ID[:, 0:3:2, :])

        rcp = wk.tile([P, F], f32, tag="rcp")
        nc.vector.reciprocal_approx_fast(rcp[:], idsel[:, 1, :])
        iou = wk.tile([P, F], f32, tag="iou")
        nc.vector.tensor_mul(iou[:], idsel[:, 0, :], rcp[:])
        nc.vector.scalar_tensor_tensor(                 # c_sel - iou
            sel[:, 4, :], iou[:], -1.0, sel[:, 4, :],
            op0=Alu.mult, op1=Alu.add)

        # ---------- masks ----------
        mo = wk.tile([P, F], f32, tag="mo")
        nc.vector.tensor_single_scalar(mo[:], tv[:, 4, :], 0.0, op=Alu.is_gt)
        mob = wk.tile([P, F], bf16, tag="mob")
        nc.vector.tensor_copy(mob[:], mo[:])
        mo25 = wk.tile([P, F], f32, tag="mo25")
        nc.scalar.mul(mo25[:], mo[:], 5.0)
        mnh = wk.tile([P, F], f32, tag="mnh")           # sqrt(.5)*(1-mo)
        nc.vector.tensor_scalar(mnh[:], mo[:], -SQH, SQH,
                                op0=Alu.mult, op1=Alu.add)

        nc.vector.tensor_mul(sel[:, 0:2, :], sel[:, 0:2, :], bc(mo[:], 2))
        nc.vector.tensor_mul(sel[:, 2:4, :], sel[:, 2:4, :], bc(mo25[:], 2))
        nc.vector.tensor_mul(sel[:, 4, :], sel[:, 4, :], mo[:])
        mtwh = wk.tile([P, 2, F], f32, tag="mtwh")
        nc.gpsimd.tensor_mul(mtwh[:], tv[:, 2:4, :], bc(mo25[:], 2))

        nc.scalar.activation(sel[:, 2:4, :], sel[:, 2:4, :], Act.Sqrt,
                             bias=eps5c[:])
        st = wk.tile([P, 2, F], f32, tag="stw")
        nc.scalar.activation(st[:], mtwh[:], Act.Sqrt, bias=eps5c[:])
        nc.vector.tensor_sub(sel[:, 2:4, :], sel[:, 2:4, :], st[:])

        # noobj conf (slots 5,6)
        nc.vector.tensor_mul(sel[:, 5:7, :], pv[:, 4:10:5, :], bc(mnh[:], 2))

        # ---------- cls (DVE bf16 2x; gpsimd only feeds the cast DMAs) -----
        mcls = wk.tile([P, 20, F], bf16, tag="mcls")
        nc.vector.tensor_sub(mcls[:], pcls[:], tcls[:])
        nc.vector.tensor_mul(mcls[:], mcls[:], bc(mob[:], 20))

        # ---------- square-accumulate (3 slots: xy*5, rest, cls) ----------
        nc.scalar.activation(sel[:, 0:2, :], sel[:, 0:2, :], Act.Square,
                             scale=SQRT5, accum_out=stats[:, 3 * t:3 * t + 1])
        nc.scalar.activation(sel[:, 2:7, :], sel[:, 2:7, :], Act.Square,
                             accum_out=stats[:, 3 * t + 1:3 * t + 2])
        nc.scalar.activation(mcls[:], mcls[:], Act.Square,
                             accum_out=stats[:, 3 * t + 2:3 * t + 3])

    total = stp.tile([P, 1], f32)
    nc.vector.tensor_reduce(total[:], stats[:], axis=mybir.AxisListType.X,
                            op=Alu.add)
    nc.sync.dma_start(out_ap, total[:])


def _build():
    if "nc" in _CACHE:
        return _CACHE["nc"]
    nc = bacc.Bacc("TRN2", target_bir_lowering=False, debug=False)
    pred = nc.dram_tensor("predicts", [P, T_TILES, C, F], f32,
                          kind="ExternalInput")
    targ = nc.dram_tensor("targets", [P, T_TILES, C, F], f32,
                          kind="ExternalInput")
    out = nc.dram_tensor("out", [P, 1], f32, kind="ExternalOutput")
    with tile.TileContext(nc) as tc, ExitStack() as ctx:
        _build_body(tc, ctx, pred.ap(), targ.ap(), out.ap())
    nc.compile()
    _CACHE["nc"] = nc
    return nc


def _shard(x):
    # [8192,7,7,30] -> per-core [P, T, C, F] channel-planar
    x = np.ascontiguousarray(x, dtype=np.float32)
    out = []
    for i in range(N_CORES):
        s = x[i * (BATCH // N_CORES):(i + 1) * (BATCH // N_CORES)]
        s = s.reshape(P, T_TILES, F, C).transpose(0, 1, 3, 2)
        out.append(np.ascontiguousarray(s))
    return out


def run(predicts, targets, trace=False, **trace_kwargs):
    nc = _build()
    pshards = _shard(predicts)
    tshards = _shard(targets)
    in_maps = [{"predicts": pshards[i], "targets": tshards[i]}
               for i in range(N_CORES)]
    res = bass_utils.run_bass_kernel_spmd(
        nc, in_maps, core_ids=list(range(N_CORES)), trace=trace,
        **trace_kwargs)
    partial = np.zeros((), dtype=np.float64)
    for r in res.results:
        partial += np.asarray(r["out"], dtype=np.float64).sum()
    return np.float32(partial), res


def kernel(predicts, targets):
    out, _ = run(predicts, targets, trace=False)
    return out



# revision 5
# speedup vs baseline: 1.1336x; 1.1336x over previous
"""YOLOv1 loss (nn_LossModul_16277926052544) on 8 TRN2 NeuronCores.

Pure data parallel: batch 8192 -> 8 shards of 1024. Each core computes a
partial loss over its shard; host sums the 8x128 partials.

v2 design (vs 75us baseline):
  * all inputs host-cast to bf16 -> HBM traffic halves (DVE TT gets 2x_1p,
    TS gets 4x). STT/copy_predicated/TTR run 1x on cayman, so the op mix
    is restructured to plain TT/TS wherever possible.
  * pred-targ diffs (xy and cls channels: 24 of 33 rows) are computed BY
    THE DMA: host stores negated targets; a SWDGE accum_op=add DMA adds
    them into the freshly-loaded pred rows (SDMA CCE inline add). The
    20-channel cls diff + 4-channel xy diff never touch a compute engine.
  * one consolidated ACT Square+accum per tile over a [P,27,F] work tile
    (all masked pre-square values); sqrt is the only other ACT work.
  * clamps via tensor_scalar max (4x) instead of ACT relu round-trips;
    |R*dxy| via ACT Abs with scale=R (free affine).
  * channel-planar [P, C, F] tiles, F contiguous.

Math restructuring (validated in the v1 session to 1.9e-4 in bf16):
  * IoU is translation invariant; per-axis overlap = min(pw, tw, s-R|dx|)
    clamped >= 0, s = (pw+tw)/2.  resp = iou1>iou2 <=> I1*D2 > I2*D1.
  * every loss term is a masked square; lambdas fold into the masks
    (5*mo, sqrt5*mo, sqrt(.5)*(1-mo)); sqrt eps-fold: sqrt(5*mo*wh+5eps)
    terms cancel when mo=0.
"""
import sys

for _p in ("/opt/trn_rl_repo",):
    if _p not in sys.path:
        sys.path.insert(0, _p)

import numpy as np
import ml_dtypes
from contextlib import ExitStack

import concourse.bass as bass  # noqa: F401  (registers engines)
from concourse import bacc, mybir
from concourse import bass_utils
import concourse.tile as tile

N_CORES = 8
BATCH = 8192
S = 7
P = 128
CELLS_PER_CORE = (BATCH // N_CORES) * S * S   # 50176
F_TOTAL = CELLS_PER_CORE // P                 # 392
T_TILES = 2
F = F_TOTAL // T_TILES                        # 196
R = 1.0 / S
EPS5 = 5e-6                                   # 5 * EPS (lambda folded)
SQRT5 = float(np.sqrt(5.0))
SQH = float(np.sqrt(0.5))

CX = 33                                       # x rows per cell
CT = 24                                       # negated-target rows per cell

f32 = mybir.dt.float32
bf16 = mybir.dt.bfloat16
Alu = mybir.AluOpType
Act = mybir.ActivationFunctionType

_CACHE = {}


def _build_body(tc, ctx, x, tn, out_ap):
    nc = tc.nc
    inpool = ctx.enter_context(tc.tile_pool(name="in", bufs=2))
    wk = ctx.enter_context(tc.tile_pool(name="wk", bufs=2))
    stp = ctx.enter_context(tc.tile_pool(name="st", bufs=1))
    stats = stp.tile([P, T_TILES], f32)
    eps5c = stp.tile([P, 1], f32)               # bias const for Sqrt
    nc.gpsimd.memset(eps5c[:], EPS5)

    def bc(ap_pf, k):
        # [P,F] -> [P,k,F] broadcast
        return ap_pf.unsqueeze(1).broadcast_to([P, k, F])

    for t in range(T_TILES):
        # x rows: 0:2 pxy1 | 2:4 pxy2 | 4:24 pcls | 24:27 pw1,ph1,pc1
        #         27:30 pw2,ph2,pc2 | 30:32 tw,th | 32 tconf
        # tn rows: 0:2 -txy | 2:4 -txy (dup) | 4:24 -tcls
        xp = inpool.tile([P, CX, F], bf16, tag="x")
        dum = inpool.tile([P, 2], bf16, tag="dum")
        # geometry-critical rows first, then the accumulating diffs, then cls
        nc.sync.dma_start(xp[:, 0:4], x[:, t, 0:4])
        nc.sync.dma_start(xp[:, 24:33], x[:, t, 24:33])
        nc.sync.dma_start(xp[:, 4:24], x[:, t, 4:24])
        # Tile does not model the accum DMA's read of its own output region,
        # so the plain load can race it.  A 1-element GpSimd read of that
        # region stalls the (in-order) GpSimd queue on the load's completion
        # sem; the accum's descriptor emission then follows it.
        nc.gpsimd.tensor_copy(dum[:, 0:1], xp[:, 0, 0:1])
        nc.gpsimd.dma_start(xp[:, 0:4], tn[:, t, 0:4], accum_op=Alu.add)
        nc.gpsimd.tensor_copy(dum[:, 1:2], xp[:, 4, 0:1])
        # CCE accumulate mangles transfers over ~2048 elems/partition; split.
        nc.gpsimd.dma_start(xp[:, 4:14], tn[:, t, 4:14], accum_op=Alu.add)
        nc.gpsimd.dma_start(xp[:, 14:24], tn[:, t, 14:24], accum_op=Alu.add)

        dxy = xp[:, 0:4].rearrange("p (b c) f -> p b c f", b=2)   # post-accum
        dcls = xp[:, 4:24]
        pbox = xp[:, 24:30].rearrange("p (b c) f -> p b c f", b=2)
        pwh = pbox[:, :, 0:2, :]                                  # [P,2,2,F]
        pc = pbox[:, :, 2, :]                                     # [P,2,F]
        twh = xp[:, 30:32]                                        # [P,2,F]
        twh_b = xp[:, 30:32].unsqueeze(1).broadcast_to([P, 2, 2, F])
        tcf = xp[:, 32]                                           # [P,F]

        # ---------------- masks (TS, 4x) ----------------
        mo = wk.tile([P, F], bf16, tag="mo")
        nc.vector.tensor_scalar(mo[:], tcf, 0.0, None, op0=Alu.is_gt)
        m5 = wk.tile([P, F], bf16, tag="m5")
        nc.vector.tensor_scalar(m5[:], tcf, 0.0, 5.0, op0=Alu.is_gt,
                                op1=Alu.mult)
        ms5 = wk.tile([P, F], bf16, tag="ms5")
        nc.vector.tensor_scalar(ms5[:], tcf, 0.0, SQRT5, op0=Alu.is_gt,
                                op1=Alu.mult)
        mnh = wk.tile([P, F], bf16, tag="mnh")
        nc.vector.tensor_scalar(mnh[:], tcf, 0.0, SQH, op0=Alu.is_le,
                                op1=Alu.mult)

        # ---------------- geometry ----------------
        absrd = wk.tile([P, 2, 2, F], bf16, tag="absrd")   # |R*dxy|
        nc.scalar.activation(absrd[:], dxy, Act.Abs, scale=R)
        sth = wk.tile([P, 2, F], bf16, tag="sth")          # twh/2
        nc.vector.tensor_scalar(sth[:], twh, 0.5, None, op0=Alu.mult)
        hp = wk.tile([P, 2, 2, F], bf16, tag="hp")         # pwh/2
        nc.vector.tensor_scalar(hp[:], pwh, 0.5, None, op0=Alu.mult)
        s = wk.tile([P, 2, 2, F], bf16, tag="s")           # (pwh+twh)/2
        nc.vector.tensor_tensor(
            s[:], hp[:], sth[:].unsqueeze(1).broadcast_to([P, 2, 2, F]),
            op=Alu.add)
        m = wk.tile([P, 2, 2, F], bf16, tag="m")           # s - R|dxy|
        nc.vector.tensor_sub(m[:], s[:], absrd[:])
        mwh = wk.tile([P, 2, 2, F], bf16, tag="mwh")       # min(pwh, twh)
        nc.vector.tensor_tensor(mwh[:], pwh, twh_b, op=Alu.min)
        ln = wk.tile([P, 2, 2, F], bf16, tag="ln")         # overlap lengths
        nc.vector.tensor_tensor(ln[:], m[:], mwh[:], op=Alu.min)
        nc.vector.tensor_scalar(ln[:], ln[:], 0.0, None, op0=Alu.max)

        ID = wk.tile([P, 4, F], bf16, tag="ID")            # I1 I2 D1 D2
        nc.vector.tensor_mul(ID[:, 0:2], ln[:, :, 0, :], ln[:, :, 1, :])
        pA = wk.tile([P, 2, F], bf16, tag="pA")
        nc.vector.tensor_mul(pA[:], pbox[:, :, 0, :], pbox[:, :, 1, :])
        tA = wk.tile([P, 1, F], bf16, tag="tA")
        nc.vector.tensor_mul(tA[:], xp[:, 30:31], xp[:, 31:32])
        PT = wk.tile([P, 2, F], bf16, tag="PT")
        nc.vector.tensor_tensor(PT[:], pA[:], tA[:].broadcast_to([P, 2, F]),
                                op=Alu.add)
        nc.vector.tensor_sub(ID[:, 2:4], PT[:], ID[:, 0:2])

        g = wk.tile([P, 2, F], bf16, tag="g")
        nc.vector.tensor_mul(g[:, 0], ID[:, 0], ID[:, 3])
        nc.vector.tensor_mul(g[:, 1], ID[:, 1], ID[:, 2])
        resp = wk.tile([P, F], mybir.dt.uint32, tag="resp")  # 1 -> box1
        nc.vector.tensor_tensor(resp[:], g[:, 0], g[:, 1], op=Alu.is_gt)

        # ---------------- select responsible box ----------------
        # sel rows: dx dy w h c I D
        sel = wk.tile([P, 7, F], bf16, tag="sel")
        nc.gpsimd.tensor_copy(sel[:, 0:2], xp[:, 2:4])     # box2 defaults
        nc.gpsimd.tensor_copy(sel[:, 2:5], xp[:, 27:30])
        nc.gpsimd.tensor_copy(sel[:, 5:7], ID[:, 1:4:2])
        nc.vector.copy_predicated(sel[:, 0:2], bc(resp[:], 2), xp[:, 0:2])
        nc.vector.copy_predicated(sel[:, 2:5], bc(resp[:], 3), xp[:, 24:27])
        nc.vector.copy_predicated(sel[:, 5:7], bc(resp[:], 2), ID[:, 0:3:2])

        # ---------------- masked pre-square values ----------------
        # W rows: 0:20 mo*dcls | 20:22 sqrt5*mo*dxy_sel | 22:24 dwh
        #         24 mo*(c_sel - iou) | 25:27 sqrt(.5)*(1-mo)*pc
        W = wk.tile([P, 27, F], bf16, tag="W")
        nc.vector.tensor_mul(W[:, 0:20], dcls, bc(mo[:], 20))
        nc.vector.tensor_mul(W[:, 20:22], sel[:, 0:2], bc(ms5[:], 2))

        sq = wk.tile([P, 4, F], bf16, tag="sq")            # w5s | tw5s
        nc.vector.tensor_mul(sq[:, 0:2], sel[:, 2:4], bc(m5[:], 2))
        nc.vector.tensor_mul(sq[:, 2:4], twh, bc(m5[:], 2))
        nc.scalar.activation(sq[:], sq[:], Act.Sqrt, bias=eps5c[:])
        nc.vector.tensor_sub(W[:, 22:24], sq[:, 0:2], sq[:, 2:4])

        Dsel = wk.tile([P, F], f32, tag="Dsel")
        nc.vector.tensor_copy(Dsel[:], sel[:, 6])
        rcp = wk.tile([P, F], f32, tag="rcp")
        nc.vector.reciprocal_approx_fast(rcp[:], Dsel[:])
        iou = wk.tile([P, F], bf16, tag="iou")
        nc.vector.tensor_mul(iou[:], sel[:, 5], rcp[:])
        cd = wk.tile([P, F], bf16, tag="cd")               # c_sel - iou
        nc.vector.tensor_sub(cd[:], sel[:, 4], iou[:])
        nc.vector.tensor_mul(W[:, 24], cd[:], mo[:])
        nc.vector.tensor_mul(W[:, 25:27], pc, bc(mnh[:], 2))

        # ---------------- square + accumulate ----------------
        nc.scalar.activation(W[:], W[:], Act.Square,
                             accum_out=stats[:, t:t + 1])

    total = stp.tile([P, 1], f32)
    nc.vector.tensor_reduce(total[:], stats[:], axis=mybir.AxisListType.X,
                            op=Alu.add)
    nc.sync.dma_start(out_ap, total[:])


def _build():
    if "nc" in _CACHE:
        return _CACHE["nc"]
    nc = bacc.Bacc("TRN2", target_bir_lowering=False, debug=False)
    x = nc.dram_tensor("x", [P, T_TILES, CX, F], bf16, kind="ExternalInput")
    tn = nc.dram_tensor("tn", [P, T_TILES, CT, F], bf16, kind="ExternalInput")
    out = nc.dram_tensor("out", [P, 1], f32, kind="ExternalOutput")
    with tile.TileContext(nc) as tc, ExitStack() as ctx:
        _build_body(tc, ctx, x.ap(), tn.ap(), out.ap())
    nc.compile()
    _CACHE["nc"] = nc
    return nc


def _shard(predicts, targets):
    """Full f32 inputs -> per-core (x, tn) bf16 arrays."""
    bpc = BATCH // N_CORES
    xs, tns = [], []
    for i in range(N_CORES):
        p = np.asarray(predicts[i * bpc:(i + 1) * bpc], dtype=np.float32)
        g = np.asarray(targets[i * bpc:(i + 1) * bpc], dtype=np.float32)
        # cells -> [P, T, F, 30]
        p = p.reshape(P, T_TILES, F, 30)
        g = g.reshape(P, T_TILES, F, 30)
        x = np.empty((P, T_TILES, CX, F), dtype=np.float32)
        pm = np.moveaxis(p, 3, 2)   # [P,T,30,F]
        gm = np.moveaxis(g, 3, 2)
        x[:, :, 0:2] = pm[:, :, 0:2]      # pxy1
        x[:, :, 2:4] = pm[:, :, 5:7]      # pxy2
        x[:, :, 4:24] = pm[:, :, 10:30]   # pcls
        x[:, :, 24:27] = pm[:, :, 2:5]    # pw1 ph1 pc1
        x[:, :, 27:30] = pm[:, :, 7:10]   # pw2 ph2 pc2
        x[:, :, 30:32] = gm[:, :, 2:4]    # tw th
        x[:, :, 32] = gm[:, :, 4]         # tconf
        tn = np.empty((P, T_TILES, CT, F), dtype=np.float32)
        tn[:, :, 0:2] = gm[:, :, 0:2]
        tn[:, :, 2:4] = gm[:, :, 0:2]
        tn[:, :, 4:24] = gm[:, :, 10:30]
        np.negative(tn, out=tn)
        xs.append(x.astype(ml_dtypes.bfloat16))
        tns.append(tn.astype(ml_dtypes.bfloat16))
    return xs, tns


def run(predicts, targets, trace=False, **trace_kwargs):
    nc = _build()
    xs, tns = _shard(predicts, targets)
    in_maps = [{"x": xs[i], "tn": tns[i]} for i in range(N_CORES)]
    res = bass_utils.run_bass_kernel_spmd(
        nc, in_maps, core_ids=list(range(N_CORES)), trace=trace,
        **trace_kwargs)
    partial = np.zeros((), dtype=np.float64)
    for r in res.results:
        partial += np.asarray(r["out"], dtype=np.float64).sum()
    return np.float32(partial), res


def kernel(predicts, targets):
    out, _ = run(predicts, targets, trace=False)
    return out


# revision 7
# speedup vs baseline: 1.2204x; 1.0765x over previous
"""YOLOv1 loss (nn_LossModul_16277926052544) on 8 TRN2 NeuronCores.

Pure data parallel: batch 8192 -> 8 shards of 1024. Each core computes a
partial loss over its shard; host sums the 8x128 partials.

v3 design (baseline 75us -> v2 62.7us -> this):
  * all inputs host-cast to bf16 -> HBM traffic halves; DVE TT gets 2x_1p,
    TS gets 4x. STT/copy_predicated run 1x on cayman, so plain TT/TS are
    preferred everywhere.
  * the 20-channel cls diff (pcls - tcls) is computed BY THE DMA: host
    stores negated targets; SWDGE accum_op=add DMAs (SDMA CCE inline add)
    add them onto the freshly-loaded pred rows.  The plain cls load is
    issued on the *gpsimd* queue so the in-order GP queue sequences the
    accumulates after it (Tile does not model the accum's RMW read).
    CCE mangles accum transfers over ~2048 elems/partition -> split.
  * xy diffs on DVE (negated txy rows ride in x): a GP round-trip costs
    more than the 2 TT ops.
  * nothing else on GpSimd: every GP op measured ~1.5-2.4us fixed cost,
    and GP compute ops block DVE 2-port (4x) ops via the shared SBUF port.
  * |R*dxy| via bf16 sign-bit AND on DVE (bitcast u16), not ACT Abs:
    avoids a V->A->V ping-pong and the second ACT table-set load.
  * ACT does only Sqrt (scale=5 folds lambda_coord) and two Square+accum
    ops per tile (cls 20 rows / rest 7 rows, so the tail op is short).
"""
import sys

for _p in ("/opt/trn_rl_repo",):
    if _p not in sys.path:
        sys.path.insert(0, _p)

import numpy as np
import ml_dtypes
from contextlib import ExitStack

import concourse.bass as bass  # noqa: F401  (registers engines)
from concourse import bacc, mybir
from concourse import bass_utils
import concourse.tile as tile

N_CORES = 8
BATCH = 8192
S = 7
P = 128
CELLS_PER_CORE = (BATCH // N_CORES) * S * S   # 50176
F_TOTAL = CELLS_PER_CORE // P                 # 392
T_TILES = 2
F = F_TOTAL // T_TILES                        # 196
R = 1.0 / S
EPS5 = 5e-6                                   # 5 * EPS (lambda folded)
SQRT5 = float(np.sqrt(5.0))
SQH = float(np.sqrt(0.5))

CX = 35                                       # x rows per cell
CT = 20                                       # negated-target cls rows

f32 = mybir.dt.float32
bf16 = mybir.dt.bfloat16
u16 = mybir.dt.uint16
Alu = mybir.AluOpType
Act = mybir.ActivationFunctionType

_CACHE = {}


def _build_body(tc, ctx, x, tn, out_ap):
    nc = tc.nc
    inpool = ctx.enter_context(tc.tile_pool(name="in", bufs=2))
    wk = ctx.enter_context(tc.tile_pool(name="wk", bufs=2))
    stp = ctx.enter_context(tc.tile_pool(name="st", bufs=1))
    stats = stp.tile([P, 2 * T_TILES], f32)
    eps5c = stp.tile([P, 1], f32)               # bias const for Sqrt
    nc.gpsimd.memset(eps5c[:], EPS5)

    def bc(ap_pf, k):
        # [P,F] -> [P,k,F] broadcast
        return ap_pf.unsqueeze(1).broadcast_to([P, k, F])

    for t in range(T_TILES):
        # x rows: 0:2 pxy1 | 2:4 pxy2 | 4:24 pcls | 24:27 pw1,ph1,pc1
        #         27:30 pw2,ph2,pc2 | 30:32 tw,th | 32 tc | 33:35 -tx,-ty
        # tn rows: 0:20 -tcls
        xp = inpool.tile([P, CX, F], bf16, tag="x")
        nc.sync.dma_start(xp[:, 0:4], x[:, t, 0:4])
        nc.sync.dma_start(xp[:, 24:35], x[:, t, 24:35])
        # cls rows ride the (in-order) gpsimd queue so the CCE accumulates
        # land after the plain load
        nc.gpsimd.dma_start(xp[:, 4:24], x[:, t, 4:24])
        nc.gpsimd.dma_start(xp[:, 4:14], tn[:, t, 0:10], accum_op=Alu.add)
        nc.gpsimd.dma_start(xp[:, 14:24], tn[:, t, 10:20], accum_op=Alu.add)

        pxy = xp[:, 0:4].rearrange("p (b c) f -> p b c f", b=2)
        dcls = xp[:, 4:24]
        pbox = xp[:, 24:30].rearrange("p (b c) f -> p b c f", b=2)
        pwh = pbox[:, :, 0:2, :]                                  # [P,2,2,F]
        pc = pbox[:, :, 2, :]                                     # [P,2,F]
        twh = xp[:, 30:32]                                        # [P,2,F]
        twh_b = xp[:, 30:32].unsqueeze(1).broadcast_to([P, 2, 2, F])
        tcf = xp[:, 32]                                           # [P,F]
        ntxy_b = xp[:, 33:35].unsqueeze(1).broadcast_to([P, 2, 2, F])

        # ---------------- masks (TS, 4x) ----------------
        mo = wk.tile([P, F], bf16, tag="mo")
        nc.vector.tensor_scalar(mo[:], tcf, 0.0, None, op0=Alu.is_gt)
        ms5 = wk.tile([P, F], bf16, tag="ms5")
        nc.vector.tensor_scalar(ms5[:], tcf, 0.0, SQRT5, op0=Alu.is_gt,
                                op1=Alu.mult)
        mnh = wk.tile([P, F], bf16, tag="mnh")
        nc.vector.tensor_scalar(mnh[:], tcf, 0.0, SQH, op0=Alu.is_le,
                                op1=Alu.mult)

        # work tile rows: 0:20 mo*dcls | 20:22 sqrt5*mo*dxy_sel | 22:24 dwh
        #                 24 mo*(c_sel-iou) | 25:27 sqrt(.5)*(1-mo)*pc
        W = wk.tile([P, 27, F], bf16, tag="W")
        nc.vector.tensor_mul(W[:, 0:20], dcls, bc(mo[:], 20))

        # ---------------- geometry ----------------
        dxy = wk.tile([P, 2, 2, F], bf16, tag="dxy")
        nc.vector.tensor_tensor(dxy[:], pxy, ntxy_b, op=Alu.add)
        absrd = wk.tile([P, 2, 2, F], bf16, tag="absrd")   # |R*dxy|
        nc.vector.tensor_scalar(absrd[:], dxy[:], R, None, op0=Alu.mult)
        nc.vector.tensor_scalar(
            absrd[:].bitcast(u16), absrd[:].bitcast(u16), 0x7FFF, None,
            op0=Alu.bitwise_and)
        sth = wk.tile([P, 2, F], bf16, tag="sth")          # twh/2
        nc.vector.tensor_scalar(sth[:], twh, 0.5, None, op0=Alu.mult)
        hp = wk.tile([P, 2, 2, F], bf16, tag="hp")         # pwh/2
        nc.vector.tensor_scalar(hp[:], pwh, 0.5, None, op0=Alu.mult)
        s = wk.tile([P, 2, 2, F], bf16, tag="s")           # (pwh+twh)/2
        nc.vector.tensor_tensor(
            s[:], hp[:], sth[:].unsqueeze(1).broadcast_to([P, 2, 2, F]),
            op=Alu.add)
        m = wk.tile([P, 2, 2, F], bf16, tag="m")           # s - R|dxy|
        nc.vector.tensor_sub(
            m[:].rearrange("p b c f -> p (b c f)"),
            s[:].rearrange("p b c f -> p (b c f)"),
            absrd[:].rearrange("p b c f -> p (b c f)"))
        mwh = wk.tile([P, 2, 2, F], bf16, tag="mwh")       # min(pwh, twh)
        nc.vector.tensor_tensor(mwh[:], pwh, twh_b, op=Alu.min)
        ln = wk.tile([P, 2, 2, F], bf16, tag="ln")         # clamped overlap
        nc.vector.scalar_tensor_tensor(
            ln[:].rearrange("p b c f -> p (b c f)"),
            m[:].rearrange("p b c f -> p (b c f)"), 0.0,
            mwh[:].rearrange("p b c f -> p (b c f)"),
            op0=Alu.max, op1=Alu.min)

        ID = wk.tile([P, 4, F], bf16, tag="ID")            # I1 I2 D1 D2
        nc.vector.tensor_mul(ID[:, 0:2], ln[:, :, 0, :], ln[:, :, 1, :])
        pA = wk.tile([P, 2, F], bf16, tag="pA")
        nc.vector.tensor_mul(pA[:], pbox[:, :, 0, :], pbox[:, :, 1, :])
        tA = wk.tile([P, 1, F], bf16, tag="tA")
        nc.vector.tensor_mul(tA[:], xp[:, 30:31], xp[:, 31:32])
        PT = wk.tile([P, 2, F], bf16, tag="PT")
        nc.vector.tensor_tensor(PT[:], pA[:], tA[:].broadcast_to([P, 2, F]),
                                op=Alu.add)
        nc.vector.tensor_sub(ID[:, 2:4], PT[:], ID[:, 0:2])

        g = wk.tile([P, 2, F], bf16, tag="g")
        nc.vector.tensor_mul(g[:, 0], ID[:, 0], ID[:, 3])
        nc.vector.tensor_mul(g[:, 1], ID[:, 1], ID[:, 2])
        resp = wk.tile([P, F], mybir.dt.uint32, tag="resp")  # 1 -> box1
        nc.vector.tensor_tensor(resp[:], g[:, 0], g[:, 1], op=Alu.is_gt)

        # ---------------- select responsible box ----------------
        # sel rows: dx dy w h c I D
        sel = wk.tile([P, 7, F], bf16, tag="sel")
        nc.vector.tensor_copy(sel[:, 0:2], dxy[:, 1])      # box2 defaults
        nc.vector.tensor_copy(sel[:, 2:5], xp[:, 27:30])
        nc.vector.tensor_copy(sel[:, 5:7], ID[:, 1:4:2])
        nc.vector.copy_predicated(sel[:, 0:2], bc(resp[:], 2), dxy[:, 0])
        nc.vector.copy_predicated(sel[:, 2:5], bc(resp[:], 3), xp[:, 24:27])
        nc.vector.copy_predicated(sel[:, 5:7], bc(resp[:], 2), ID[:, 0:3:2])

        # ---------------- masked pre-square values ----------------
        nc.vector.tensor_mul(W[:, 20:22], sel[:, 0:2], bc(ms5[:], 2))

        sq = wk.tile([P, 4, F], bf16, tag="sq")            # mo*selwh | mo*twh
        nc.vector.tensor_mul(sq[:, 0:2], sel[:, 2:4], bc(mo[:], 2))
        nc.vector.tensor_mul(sq[:, 2:4], twh, bc(mo[:], 2))
        # sqrt(5*mo*wh + 5eps): lambda_coord folds into the free affine
        nc.scalar.activation(sq[:], sq[:], Act.Sqrt, bias=eps5c[:], scale=5.0)
        nc.vector.tensor_sub(W[:, 22:24], sq[:, 0:2], sq[:, 2:4])

        Dsel = wk.tile([P, F], f32, tag="Dsel")
        nc.vector.tensor_copy(Dsel[:], sel[:, 6])
        rcp = wk.tile([P, F], f32, tag="rcp")
        nc.vector.reciprocal_approx_fast(rcp[:], Dsel[:])
        iou = wk.tile([P, F], bf16, tag="iou")
        nc.vector.tensor_mul(iou[:], sel[:, 5], rcp[:])
        cd = wk.tile([P, F], bf16, tag="cd")               # c_sel - iou
        nc.vector.tensor_sub(cd[:], sel[:, 4], iou[:])
        nc.vector.tensor_mul(W[:, 24], cd[:], mo[:])
        nc.vector.tensor_mul(W[:, 25:27], pc, bc(mnh[:], 2))

        # ---------------- square + accumulate ----------------
        nc.scalar.activation(W[:, 0:20], W[:, 0:20], Act.Square,
                             accum_out=stats[:, 2 * t:2 * t + 1])
        nc.scalar.activation(W[:, 20:27], W[:, 20:27], Act.Square,
                             accum_out=stats[:, 2 * t + 1:2 * t + 2])

    total = stp.tile([P, 1], f32)
    nc.vector.tensor_reduce(total[:], stats[:], axis=mybir.AxisListType.X,
                            op=Alu.add)
    nc.sync.dma_start(out_ap, total[:])


def _build():
    if "nc" in _CACHE:
        return _CACHE["nc"]
    nc = bacc.Bacc("TRN2", target_bir_lowering=False, debug=False)
    x = nc.dram_tensor("x", [P, T_TILES, CX, F], bf16, kind="ExternalInput")
    tn = nc.dram_tensor("tn", [P, T_TILES, CT, F], bf16, kind="ExternalInput")
    out = nc.dram_tensor("out", [P, 1], f32, kind="ExternalOutput")
    with tile.TileContext(nc) as tc, ExitStack() as ctx:
        _build_body(tc, ctx, x.ap(), tn.ap(), out.ap())
    nc.compile()
    _CACHE["nc"] = nc
    return nc


def _shard(predicts, targets):
    """Full f32 inputs -> per-core (x, tn) bf16 arrays."""
    bpc = BATCH // N_CORES
    xs, tns = [], []
    for i in range(N_CORES):
        p = np.asarray(predicts[i * bpc:(i + 1) * bpc], dtype=np.float32)
        g = np.asarray(targets[i * bpc:(i + 1) * bpc], dtype=np.float32)
        p = p.reshape(P, T_TILES, F, 30)
        g = g.reshape(P, T_TILES, F, 30)
        pm = np.moveaxis(p, 3, 2)   # [P,T,30,F]
        gm = np.moveaxis(g, 3, 2)
        x = np.empty((P, T_TILES, CX, F), dtype=np.float32)
        x[:, :, 0:2] = pm[:, :, 0:2]      # pxy1
        x[:, :, 2:4] = pm[:, :, 5:7]      # pxy2
        x[:, :, 4:24] = pm[:, :, 10:30]   # pcls
        x[:, :, 24:27] = pm[:, :, 2:5]    # pw1 ph1 pc1
        x[:, :, 27:30] = pm[:, :, 7:10]   # pw2 ph2 pc2
        x[:, :, 30:32] = gm[:, :, 2:4]    # tw th
        x[:, :, 32] = gm[:, :, 4]         # tconf
        x[:, :, 33:35] = -gm[:, :, 0:2]   # -tx -ty
        tn = -gm[:, :, 10:30]             # -tcls
        xs.append(x.astype(ml_dtypes.bfloat16))
        tns.append(np.ascontiguousarray(tn).astype(ml_dtypes.bfloat16))
    return xs, tns


def run(predicts, targets, trace=False, **trace_kwargs):
    nc = _build()
    xs, tns = _shard(predicts, targets)
    in_maps = [{"x": xs[i], "tn": tns[i]} for i in range(N_CORES)]
    res = bass_utils.run_bass_kernel_spmd(
        nc, in_maps, core_ids=list(range(N_CORES)), trace=trace,
        **trace_kwargs)
    partial = np.zeros((), dtype=np.float64)
    for r in res.results:
        partial += np.asarray(r["out"], dtype=np.float64).sum()
    return np.float32(partial), res


def kernel(predicts, targets):
    out, _ = run(predicts, targets, trace=False)
    return out


# revision 8
# speedup vs baseline: 1.2369x; 1.0135x over previous
"""YOLOv1 loss (nn_LossModul_16277926052544) on 8 TRN2 NeuronCores.

Pure data parallel: batch 8192 -> 8 shards of 1024. Each core computes a
partial loss over its shard; host sums the 8x128 partials.

v4 design (75us baseline -> 62.7 -> 58.3 -> this). Findings driving it:
  * ~350ns fixed cost per Vector op (SBUF access init + a ~200ns
    semaphore instr) dominated v3: 65 ops x 350ns ~ 23us of overhead vs
    ~15us of actual streaming.  One 392-cell tile (T=1) halves the op
    count; the fixed ~10us NEFF teardown also shrinks with sem count.
  * all inputs host-cast bf16: HBM halves; TT 2x_1p, TS 4x.  STT and
    copy_predicated run 1x on cayman -> plain TT/TS preferred.
  * cls diff (pcls-tcls, 20 of 35 rows) computed BY THE DMA: host stores
    negated targets, SWDGE accum_op=add DMAs (SDMA CCE) add them onto the
    loaded pred rows.  The plain cls load rides the same in-order gpsimd
    queue; CCE mangles >2048 elems/partition per transfer -> 5-row chunks.
  * geometry rows packed contiguous (one early HWDGE DMA) so Vector
    starts ~5us sooner; cls streams in behind it.
  * no GpSimd compute at all (each GP op ~1.5-2us fixed + shared-port
    conflicts with DVE 4x ops).
  * |R*dxy| via bf16 sign-bit AND (bitcast u16 TS) -- no ACT round-trip.
  * a warm-up Sqrt makes ACT load its single table set (sqrt set also
    contains Square) during the DMA ramp instead of mid-pipeline.
  * ACT does Sqrt (scale=5 folds lambda) + three Square+accum ops sized
    so they pipeline behind Vector and the tail op is short.
"""
import sys

for _p in ("/opt/trn_rl_repo",):
    if _p not in sys.path:
        sys.path.insert(0, _p)

import numpy as np
import ml_dtypes
from contextlib import ExitStack

import concourse.bass as bass  # noqa: F401  (registers engines)
from concourse import bacc, mybir
from concourse import bass_utils
import concourse.tile as tile

N_CORES = 8
BATCH = 8192
S = 7
P = 128
F = (BATCH // N_CORES) * S * S // P           # 392 cells per partition
R = 1.0 / S
EPS5 = 5e-6                                   # 5 * EPS (lambda folded)
SQRT5 = float(np.sqrt(5.0))
SQH = float(np.sqrt(0.5))

CX = 35                                       # x rows per cell
CT = 20                                       # negated-target cls rows

f32 = mybir.dt.float32
bf16 = mybir.dt.bfloat16
u16 = mybir.dt.uint16
u32 = mybir.dt.uint32
Alu = mybir.AluOpType
Act = mybir.ActivationFunctionType

_CACHE = {}


def _build_body(tc, ctx, x, tn, out_ap):
    nc = tc.nc
    wk = ctx.enter_context(tc.tile_pool(name="wk", bufs=1))
    stats = wk.tile([P, 4], f32)
    eps5c = wk.tile([P, 1], f32)                # bias const for Sqrt
    nc.gpsimd.memset(eps5c[:], EPS5)
    warm = wk.tile([P, 1], f32)                 # pulls the sqrt table set in
    nc.scalar.activation(warm[:], eps5c[:], Act.Sqrt)

    # x rows: 0:2 pxy1 | 2:4 pxy2 | 4:7 pw1,ph1,pc1 | 7:10 pw2,ph2,pc2
    #         10:12 tw,th | 12 tc | 13:15 -tx,-ty | 15:35 pcls
    # tn rows: 0:20 -tcls
    xp = wk.tile([P, CX, F], bf16, tag="x")
    nc.sync.dma_start(xp[:, 0:15], x[:, 0:15])
    # cls rows ride the (in-order) gpsimd queue so the CCE accumulates land
    # after the plain load; accum chunks stay under the ~2048 elem/partition
    # CCE limit (5 rows * 392 = 1960)
    nc.gpsimd.dma_start(xp[:, 15:35], x[:, 15:35])
    for k in range(4):
        nc.gpsimd.dma_start(xp[:, 15 + 5 * k:20 + 5 * k],
                            tn[:, 5 * k:5 * k + 5], accum_op=Alu.add)

    pxy = xp[:, 0:4].rearrange("p (b c) f -> p b c f", b=2)
    pbox = xp[:, 4:10].rearrange("p (b c) f -> p b c f", b=2)
    pwh = pbox[:, :, 0:2, :]                                  # [P,2,2,F]
    pc = pbox[:, :, 2, :]                                     # [P,2,F]
    twh = xp[:, 10:12]                                        # [P,2,F]
    twh_b = xp[:, 10:12].unsqueeze(1).broadcast_to([P, 2, 2, F])
    tcf = xp[:, 12]                                           # [P,F]
    ntxy_b = xp[:, 13:15].unsqueeze(1).broadcast_to([P, 2, 2, F])
    dcls = xp[:, 15:35]

    def bc(ap_pf, k):
        return ap_pf.unsqueeze(1).broadcast_to([P, k, F])

    flat = lambda a: a.rearrange("p b c f -> p (b c f)")

    # ---------------- masks (TS, 4x) ----------------
    mo = wk.tile([P, F], bf16, tag="mo")
    nc.vector.tensor_scalar(mo[:], tcf, 0.0, None, op0=Alu.is_gt)
    ms5 = wk.tile([P, F], bf16, tag="ms5")
    nc.vector.tensor_scalar(ms5[:], tcf, 0.0, SQRT5, op0=Alu.is_gt,
                            op1=Alu.mult)
    mnh = wk.tile([P, F], bf16, tag="mnh")
    nc.vector.tensor_scalar(mnh[:], tcf, 0.0, SQH, op0=Alu.is_le,
                            op1=Alu.mult)

    # ---------------- geometry ----------------
    dxy = wk.tile([P, 2, 2, F], bf16, tag="dxy")
    nc.vector.tensor_tensor(dxy[:], pxy, ntxy_b, op=Alu.add)
    absrd = wk.tile([P, 2, 2, F], bf16, tag="absrd")   # |R*dxy|
    nc.vector.tensor_scalar(flat(absrd[:]), flat(dxy[:]), R, None,
                            op0=Alu.mult)
    nc.vector.tensor_scalar(
        flat(absrd[:]).bitcast(u16), flat(absrd[:]).bitcast(u16), 0x7FFF,
        None, op0=Alu.bitwise_and)
    sth = wk.tile([P, 2, F], bf16, tag="sth")          # twh/2
    nc.vector.tensor_scalar(sth[:], twh, 0.5, None, op0=Alu.mult)
    hp = wk.tile([P, 2, 2, F], bf16, tag="hp")         # pwh/2
    nc.vector.tensor_scalar(hp[:], pwh, 0.5, None, op0=Alu.mult)
    s = wk.tile([P, 2, 2, F], bf16, tag="s")           # (pwh+twh)/2
    nc.vector.tensor_tensor(
        s[:], hp[:], sth[:].unsqueeze(1).broadcast_to([P, 2, 2, F]),
        op=Alu.add)
    m = wk.tile([P, 2, 2, F], bf16, tag="m")           # s - R|dxy|
    nc.vector.tensor_sub(flat(m[:]), flat(s[:]), flat(absrd[:]))
    mwh = wk.tile([P, 2, 2, F], bf16, tag="mwh")       # min(pwh, twh)
    nc.vector.tensor_tensor(mwh[:], pwh, twh_b, op=Alu.min)
    ln = wk.tile([P, 2, 2, F], bf16, tag="ln")         # clamped overlap
    nc.vector.scalar_tensor_tensor(flat(ln[:]), flat(m[:]), 0.0,
                                   flat(mwh[:]), op0=Alu.max, op1=Alu.min)

    ID = wk.tile([P, 4, F], bf16, tag="ID")            # I1 I2 D1 D2
    nc.vector.tensor_mul(ID[:, 0:2], ln[:, :, 0, :], ln[:, :, 1, :])
    pA = wk.tile([P, 2, F], bf16, tag="pA")
    nc.vector.tensor_mul(pA[:], pbox[:, :, 0, :], pbox[:, :, 1, :])
    tA = wk.tile([P, 1, F], bf16, tag="tA")
    nc.vector.tensor_mul(tA[:], xp[:, 10:11], xp[:, 11:12])
    PT = wk.tile([P, 2, F], bf16, tag="PT")
    nc.vector.tensor_tensor(PT[:], pA[:], tA[:].broadcast_to([P, 2, F]),
                            op=Alu.add)
    nc.vector.tensor_sub(ID[:, 2:4], PT[:], ID[:, 0:2])

    g = wk.tile([P, 2, F], bf16, tag="g")
    nc.vector.tensor_mul(g[:, 0], ID[:, 0], ID[:, 3])
    nc.vector.tensor_mul(g[:, 1], ID[:, 1], ID[:, 2])
    resp = wk.tile([P, F], u32, tag="resp")            # 1 -> box1
    nc.vector.tensor_tensor(resp[:], g[:, 0], g[:, 1], op=Alu.is_gt)

    # ---------------- select responsible box ----------------
    # sel rows: dx dy w h c I D
    sel = wk.tile([P, 7, F], bf16, tag="sel")
    nc.vector.tensor_copy(sel[:, 0:2], dxy[:, 1])      # box2 defaults
    nc.vector.tensor_copy(sel[:, 2:5], xp[:, 7:10])
    nc.vector.tensor_copy(sel[:, 5:7], ID[:, 1:4:2])
    nc.vector.copy_predicated(sel[:, 0:2], bc(resp[:], 2), dxy[:, 0])
    nc.vector.copy_predicated(sel[:, 2:5], bc(resp[:], 3), xp[:, 4:7])
    nc.vector.copy_predicated(sel[:, 5:7], bc(resp[:], 2), ID[:, 0:3:2])

    # work tile rows: 0:20 mo*dcls | 20:22 sqrt5*mo*dxy_sel | 22:24 dwh
    #                 24 mo*(c_sel-iou) | 25:27 sqrt(.5)*(1-mo)*pc
    W = wk.tile([P, 27, F], bf16, tag="W")
    nc.vector.tensor_mul(W[:, 20:22], sel[:, 0:2], bc(ms5[:], 2))

    sq = wk.tile([P, 4, F], bf16, tag="sq")            # mo*selwh | mo*twh
    nc.vector.tensor_mul(sq[:, 0:2], sel[:, 2:4], bc(mo[:], 2))
    nc.vector.tensor_mul(sq[:, 2:4], twh, bc(mo[:], 2))
    # sqrt(5*mo*wh + 5eps): lambda_coord folds into the free affine
    nc.scalar.activation(sq[:], sq[:], Act.Sqrt, bias=eps5c[:], scale=5.0)
    nc.vector.tensor_sub(W[:, 22:24], sq[:, 0:2], sq[:, 2:4])

    Dsel = wk.tile([P, F], f32, tag="Dsel")
    nc.vector.tensor_copy(Dsel[:], sel[:, 6])
    rcp = wk.tile([P, F], f32, tag="rcp")
    nc.vector.reciprocal_approx_fast(rcp[:], Dsel[:])
    iou = wk.tile([P, F], bf16, tag="iou")
    nc.vector.tensor_mul(iou[:], sel[:, 5], rcp[:])
    cd = wk.tile([P, F], bf16, tag="cd")               # c_sel - iou
    nc.vector.tensor_sub(cd[:], sel[:, 4], iou[:])
    nc.vector.tensor_mul(W[:, 24], cd[:], mo[:])
    nc.vector.tensor_mul(W[:, 25:27], pc, bc(mnh[:], 2))

    # cls last: its DMA-computed diffs stream in while geometry runs
    nc.vector.tensor_mul(W[:, 0:10], dcls[:, 0:10], bc(mo[:], 10))
    nc.vector.tensor_mul(W[:, 10:20], dcls[:, 10:20], bc(mo[:], 10))

    # ---------------- square + accumulate ----------------
    nc.scalar.activation(W[:, 20:27], W[:, 20:27], Act.Square,
                         accum_out=stats[:, 0:1])
    nc.scalar.activation(W[:, 0:10], W[:, 0:10], Act.Square,
                         accum_out=stats[:, 1:2])
    nc.scalar.activation(W[:, 10:20], W[:, 10:20], Act.Square,
                         accum_out=stats[:, 2:3])
    nc.vector.tensor_copy(stats[:, 3:4], warm[:])      # keep warm read live
    total = wk.tile([P, 1], f32)
    nc.vector.tensor_reduce(total[:], stats[:, 0:3],
                            axis=mybir.AxisListType.X, op=Alu.add)
    nc.sync.dma_start(out_ap, total[:])


def _build():
    if "nc" in _CACHE:
        return _CACHE["nc"]
    nc = bacc.Bacc("TRN2", target_bir_lowering=False, debug=False)
    x = nc.dram_tensor("x", [P, CX, F], bf16, kind="ExternalInput")
    tn = nc.dram_tensor("tn", [P, CT, F], bf16, kind="ExternalInput")
    out = nc.dram_tensor("out", [P, 1], f32, kind="ExternalOutput")
    with tile.TileContext(nc) as tc, ExitStack() as ctx:
        _build_body(tc, ctx, x.ap(), tn.ap(), out.ap())
    nc.compile()
    _CACHE["nc"] = nc
    return nc


def _shard(predicts, targets):
    """Full f32 inputs -> per-core (x, tn) bf16 arrays."""
    bpc = BATCH // N_CORES
    xs, tns = [], []
    for i in range(N_CORES):
        p = np.asarray(predicts[i * bpc:(i + 1) * bpc], dtype=np.float32)
        g = np.asarray(targets[i * bpc:(i + 1) * bpc], dtype=np.float32)
        pm = np.moveaxis(p.reshape(P, F, 30), 2, 1)   # [P,30,F]
        gm = np.moveaxis(g.reshape(P, F, 30), 2, 1)
        x = np.empty((P, CX, F), dtype=np.float32)
        x[:, 0:2] = pm[:, 0:2]      # pxy1
        x[:, 2:4] = pm[:, 5:7]      # pxy2
        x[:, 4:7] = pm[:, 2:5]      # pw1 ph1 pc1
        x[:, 7:10] = pm[:, 7:10]    # pw2 ph2 pc2
        x[:, 10:12] = gm[:, 2:4]    # tw th
        x[:, 12] = gm[:, 4]         # tconf
        x[:, 13:15] = -gm[:, 0:2]   # -tx -ty
        x[:, 15:35] = pm[:, 10:30]  # pcls
        tn = -gm[:, 10:30]          # -tcls
        xs.append(x.astype(ml_dtypes.bfloat16))
        tns.append(np.ascontiguousarray(tn).astype(ml_dtypes.bfloat16))
    return xs, tns


def run(predicts, targets, trace=False, **trace_kwargs):
    nc = _build()
    xs, tns = _shard(predicts, targets)
    in_maps = [{"x": xs[i], "tn": tns[i]} for i in range(N_CORES)]
    res = bass_utils.run_bass_kernel_spmd(
        nc, in_maps, core_ids=list(range(N_CORES)), trace=trace,
        **trace_kwargs)
    partial = np.zeros((), dtype=np.float64)
    for r in res.results:
        partial += np.asarray(r["out"], dtype=np.float64).sum()
    return np.float32(partial), res


def kernel(predicts, targets):
    out, _ = run(predicts, targets, trace=False)
    return out
